# revision 1
# baseline (speedup 1.0000x reference)
"""Trainium2 Bass kernel for a 2-layer GCN (EnhancedHockeyGNN).

Strategy (8 NeuronCores, SPMD, two NEFF launches):
  - Nodes sharded row-wise across cores (dst ownership); weights replicated.
  - Per layer: xs = (x @ W) * dinv computed on the owning core in fp32 and
    staged to DRAM as an fp16 message table, AllGather'd (in 4 overlapped
    chunks) into every core's DRAM.
  - Edges (incl. explicit self-loops) are sharded by dst owner and bin-packed
    into groups of <=128 dst nodes / <=2048 edges. Per 128-edge tile the src
    rows are fetched with an indirect DMA (one row per partition); a one-hot
    matrix (value dinv[dst], built wide on the vector engine) turns the
    segment-sum into a PSUM-accumulated fp16 matmul chain producing
    feature-major aggregates.
  - BN+ReLU (eval) folds into one scalar-engine activation per group.
  - Layer-2's message table is produced and AllGather'd inside part A (hidden
    under the layer-1 gather stream) and handed to part B as a plain input,
    so part B starts gathering immediately.
  - Readout computes log-softmax for every node; the host selects the
    requested game_indices rows (pure index routing).
"""
import math

import numpy as np

# ---------------------------------------------------------------- constants
N = 100000
F_IN = 128
H = 128
NC = 8
SHARD = 12544            # multiple of 128; 8 * 12544 = 100352 >= N
NPAD = NC * SHARD
GROUP_EDGES = 2048       # edges per group (16 tiles of 128)
GROUP_TILES = GROUP_EDGES // 128
GROUP_DSTS = 128         # max dst nodes per group (PSUM partition dim)
NCHUNK = 4               # AllGather overlap chunks
EPS = 1e-5

_CACHE = {}


def _chunks(n, k):
    """Split n items into k nearly-equal contiguous chunks: list of (lo, hi)."""
    k = min(k, n)
    base = n // k
    rem = n % k
    out = []
    lo = 0
    for i in range(k):
        hi = lo + base + (1 if i < rem else 0)
        out.append((lo, hi))
        lo = hi
    return out


# ---------------------------------------------------------------- host prep
def _bin_pack(counts, G):
    order = np.argsort(-counts, kind="stable")
    bin_edges = np.zeros(G, dtype=np.int64)
    bin_nodes = np.zeros(G, dtype=np.int64)
    group_of = np.full(counts.shape[0], -1, dtype=np.int32)
    pos_in_group = np.full(counts.shape[0], -1, dtype=np.int32)
    for d in order:
        c = counts[d]
        placed = False
        for b in range(G):
            if bin_edges[b] + c <= GROUP_EDGES and bin_nodes[b] < GROUP_DSTS:
                group_of[d] = b
                pos_in_group[d] = bin_nodes[b]
                bin_edges[b] += c
                bin_nodes[b] += 1
                placed = True
                break
        if not placed:
            return None
    return group_of, pos_in_group


def _prepare(x, edge_index, cfg):
    n, npad, shard, nc = cfg["N"], cfg["NPAD"], cfg["SHARD"], cfg["NC"]
    ge, gt = cfg["GROUP_EDGES"], cfg["GROUP_TILES"]

    src = np.asarray(edge_index[0], dtype=np.int64)
    dst = np.asarray(edge_index[1], dtype=np.int64)
    deg = np.bincount(dst, minlength=n).astype(np.float64) + 1.0
    dinv = (1.0 / np.sqrt(deg)).astype(np.float32)
    dinv_pad_full = np.ones(npad, dtype=np.float32)
    dinv_pad_full[:n] = dinv

    sall = np.concatenate([src, np.arange(n, dtype=np.int64)])
    dall = np.concatenate([dst, np.arange(n, dtype=np.int64)])
    owner = dall // shard

    Es = [int((owner == c).sum()) for c in range(nc)]
    G = max(int(math.ceil(e / ge)) for e in Es)
    while True:
        packs = []
        ok = True
        for c in range(nc):
            m = owner == c
            d0 = (dall[m] - c * shard).astype(np.int64)
            counts = np.bincount(d0, minlength=shard)
            r = _bin_pack(counts, G)
            if r is None:
                ok = False
                break
            packs.append((r[0], r[1], d0, sall[m]))
        if ok:
            break
        G += 1

    ntiles = G * gt
    ntile_nat = shard // 128
    nchunk = cfg["NCHUNK"]

    # ----- chunk-major table layouts (AllGather per chunk)
    ch_a = _chunks(ntile_nat, nchunk)
    chunk_base_a = []
    acc = 0
    for lo, hi in ch_a:
        chunk_base_a.append(acc)
        acc += nc * (hi - lo) * 128
    tile_q = np.zeros(ntile_nat, dtype=np.int64)
    for q, (lo, hi) in enumerate(ch_a):
        tile_q[lo:hi] = q
    nodes = np.arange(npad, dtype=np.int64)
    c_of = nodes // shard
    loc = nodes % shard
    j_of = loc // 128
    p_of = loc % 128
    q_of = tile_q[j_of]
    rows_q = np.array([hi - lo for lo, hi in ch_a], dtype=np.int64)[q_of] * 128
    lo_q = np.array([lo for lo, hi in ch_a], dtype=np.int64)[q_of]
    base_q = np.array(chunk_base_a, dtype=np.int64)[q_of]
    xs1_row = base_q + c_of * rows_q + (j_of - lo_q) * 128 + p_of

    ch_d = _chunks(G, nchunk)
    chunk_base_d = []
    acc = 0
    for lo, hi in ch_d:
        chunk_base_d.append(acc)
        acc += nc * (hi - lo) * 128
    g_q = np.zeros(G, dtype=np.int64)
    for q, (lo, hi) in enumerate(ch_d):
        g_q[lo:hi] = q

    padded_row = np.zeros(npad, dtype=np.int64)      # node -> xs2 row
    pad_cji = np.zeros((npad, 3), dtype=np.int64)    # node -> (core, group, pos)
    for c in range(nc):
        group_of, pos, _, _ = packs[c]
        g64 = group_of.astype(np.int64)
        p64 = pos.astype(np.int64)
        q = g_q[g64]
        lo = np.array([l for l, _ in ch_d], dtype=np.int64)[q]
        hi = np.array([h_ for _, h_ in ch_d], dtype=np.int64)[q]
        base = np.array(chunk_base_d, dtype=np.int64)[q]
        rows = c * (hi - lo) * 128 + (g64 - lo) * 128 + p64 + base
        padded_row[c * shard:(c + 1) * shard] = rows
        pad_cji[c * shard:(c + 1) * shard, 0] = c
        pad_cji[c * shard:(c + 1) * shard, 1] = g64
        pad_cji[c * shard:(c + 1) * shard, 2] = p64

    per_core = []
    for c in range(nc):
        group_of, pos, d0, s_nodes = packs[c]
        g_of_edge = group_of[d0]
        order = np.argsort(g_of_edge, kind="stable")
        d0o, so, go = d0[order], s_nodes[order], g_of_edge[order]
        src1 = np.zeros((128, ntiles), dtype=np.int32)
        src2 = np.zeros((128, ntiles), dtype=np.int32)
        dloc = np.full((128, ntiles), 300.0, dtype=np.float16)
        dinv_dst = np.zeros((128, ntiles), dtype=np.float16)
        gstart = np.searchsorted(go, np.arange(G))
        gend = np.searchsorted(go, np.arange(G) + 1)
        for g in range(G):
            a, b = int(gstart[g]), int(gend[g])
            k = b - a
            assert k <= ge
            sl_s = so[a:b]
            sl_d = d0o[a:b]
            t = np.arange(k) // 128
            p = np.arange(k) % 128
            cols = g * gt + t
            src1[p, cols] = xs1_row[sl_s]
            src2[p, cols] = padded_row[sl_s]
            dloc[p, cols] = pos[sl_d]
            dinv_dst[p, cols] = dinv_pad_full[c * shard + sl_d]
        jj = np.arange(shard)
        dinv_nat = dinv_pad_full[c * shard + jj].reshape(shard // 128, 128).T.copy()
        xs_shape = np.zeros((shard, x.shape[1]), dtype=np.float32)
        lo, hi = c * shard, min((c + 1) * shard, n)
        xs_shape[: hi - lo] = x[lo:hi]
        xT = np.ascontiguousarray(xs_shape.T)
        inv_nodes = np.full(G * 128, -1, dtype=np.int64)
        inv_nodes[group_of.astype(np.int64) * 128 + pos.astype(np.int64)] = \
            np.arange(shard)
        valid = inv_nodes >= 0
        vals = np.zeros(G * 128, dtype=np.float32)
        vals[valid] = dinv_pad_full[c * shard + inv_nodes[valid]]
        dinv_padlay = vals.reshape(G, 128).T.copy()
        per_core.append(dict(src1=src1, src2=src2, dloc=dloc, dinv_dst=dinv_dst,
                             dinv_nat=dinv_nat, dinv_padlay=dinv_padlay, xT=xT))
    meta = dict(ch_a=ch_a, ch_d=ch_d, pad_cji=pad_cji)
    return per_core, meta, G, ntiles


def _fold_bn(gamma, beta, mean, var, b):
    s = (gamma / np.sqrt(var + EPS)).astype(np.float32)
    t = ((b - mean) * s + beta).astype(np.float32)
    return s.reshape(H, 1), t.reshape(H, 1)


# ---------------------------------------------------------------- bass build
def _build(cfg, G, ntiles, part, meta):
    import concourse.bacc as bacc
    import concourse.bass as bass
    import concourse.mybir as mybir
    import concourse.tile as tile

    fp32 = mybir.dt.float32
    fp16 = mybir.dt.float16
    i32 = mybir.dt.int32
    AF = mybir.ActivationFunctionType

    nc_ = cfg["NC"]
    shard = cfg["SHARD"]
    gt = cfg["GROUP_TILES"]
    ntile_nat = shard // 128
    h = cfg["H"]
    fin = cfg["F_IN"]
    ch_a = meta["ch_a"]
    ch_d = meta["ch_d"]
    xs1_rows = nc_ * ntile_nat * 128
    xs2_rows = nc_ * G * 128

    nc = bacc.Bacc(None, target_bir_lowering=False, debug=False, num_devices=nc_)

    iota_in = nc.dram_tensor("iota", [128, 128], fp16, kind="ExternalInput")
    dloc_in = nc.dram_tensor("dloc", [128, ntiles], fp16, kind="ExternalInput")
    dd_in = nc.dram_tensor("dinv_dst", [128, ntiles], fp16, kind="ExternalInput")

    if part == "a":
        xT_in = nc.dram_tensor("xT", [fin, shard], fp32, kind="ExternalInput")
        w1_in = nc.dram_tensor("W1", [fin, h], fp32, kind="ExternalInput")
        w2_in = nc.dram_tensor("W2", [h, h], fp32, kind="ExternalInput")
        s1_in = nc.dram_tensor("s1", [h, 1], fp32, kind="ExternalInput")
        t1_in = nc.dram_tensor("t1", [h, 1], fp32, kind="ExternalInput")
        src1_in = nc.dram_tensor("src1", [128, ntiles], i32, kind="ExternalInput")
        dn_in = nc.dram_tensor("dinv_nat", [128, ntile_nat], fp32,
                               kind="ExternalInput")
        dp_in = nc.dram_tensor("dinv_padlay", [128, G], fp32,
                               kind="ExternalInput")
        out_xs2 = nc.dram_tensor("xs2_full_out", [xs2_rows, h], fp16,
                                 kind="ExternalOutput")
    else:
        xs2_in = nc.dram_tensor("xs2_full_in", [xs2_rows, h], fp16,
                                kind="ExternalInput")
        wf_in = nc.dram_tensor("Wf", [h, 2], fp32, kind="ExternalInput")
        bf_in = nc.dram_tensor("bf_rep", [128, 2], fp32, kind="ExternalInput")
        s2_in = nc.dram_tensor("s2", [h, 1], fp32, kind="ExternalInput")
        t2_in = nc.dram_tensor("t2", [h, 1], fp32, kind="ExternalInput")
        src2_in = nc.dram_tensor("src2", [128, ntiles], i32, kind="ExternalInput")
        out_lp = nc.dram_tensor("logp", [128, 2 * G], fp32, kind="ExternalOutput")

    with tile.TileContext(nc) as tc:
        with (
            tc.tile_pool(name="res", bufs=1) as res,
            tc.tile_pool(name="big", bufs=1) as big,
            tc.tile_pool(name="stream", bufs=1) as st,
            tc.tile_pool(name="ps", bufs=1, space="PSUM") as ps,
            tc.tile_pool(name="dram", bufs=1, space="DRAM") as dram,
        ):
            iota_t = res.tile([128, 128], fp16)
            dloc_t = res.tile([128, ntiles], fp16)
            dd_t = res.tile([128, ntiles], fp16)
            nc.sync.dma_start(out=iota_t[:], in_=iota_in[:])
            nc.sync.dma_start(out=dloc_t[:], in_=dloc_in[:])
            nc.sync.dma_start(out=dd_t[:], in_=dd_in[:])

            def edge_layer(src_t, xs_full_ap, s_t, t_t, hT, post_group=None):
                for g in range(G):
                    oh = st.tile([128, gt, 128], fp16, name="oh", tag="oh",
                                 bufs=3)
                    nc.vector.tensor_tensor(
                        out=oh[:],
                        in0=dloc_t[:, g * gt:(g + 1) * gt].to_broadcast(
                            [128, gt, 128]),
                        in1=bass.AP(iota_t[:].tensor, iota_t[:].offset,
                                    [iota_t[:].ap[0], [0, gt], [1, 128]]),
                        op=mybir.AluOpType.is_equal,
                    )
                    nc.vector.tensor_tensor(
                        out=oh[:],
                        in0=oh[:],
                        in1=dd_t[:, g * gt:(g + 1) * gt].to_broadcast(
                            [128, gt, 128]),
                        op=mybir.AluOpType.mult,
                    )
                    pg = ps.tile([h, 128], fp32, name="pg", tag="pg", bufs=4)
                    for t in range(gt):
                        k = g * gt + t
                        msg = st.tile([128, h], fp16, name="msg", tag="msg",
                                      bufs=16)
                        nc.gpsimd.indirect_dma_start(
                            out=msg[:],
                            out_offset=None,
                            in_=xs_full_ap,
                            in_offset=bass.IndirectOffsetOnAxis(
                                ap=src_t[:, k:k + 1], axis=0),
                        )
                        nc.tensor.matmul(pg[:], msg[:], oh[:, t, :],
                                         start=(t == 0), stop=(t == gt - 1))
                    nc.scalar.activation(
                        out=hT[:, g * 128:(g + 1) * 128], in_=pg[:],
                        func=AF.Relu, bias=t_t[:], scale=s_t[:],
                    )
                    if post_group is not None:
                        post_group(g)

            if part == "a":
                w1_t = res.tile([fin, h], fp32)
                w2_t = res.tile([h, h], fp32)
                s1_t = res.tile([h, 1], fp32)
                t1_t = res.tile([h, 1], fp32)
                src1_t = res.tile([128, ntiles], i32)
                dn_t = res.tile([128, ntile_nat], fp32)
                dp_t = res.tile([128, G], fp32)
                for t_, i_ in ((w1_t, w1_in), (w2_t, w2_in), (s1_t, s1_in),
                               (t1_t, t1_in), (src1_t, src1_in), (dn_t, dn_in),
                               (dp_t, dp_in)):
                    nc.sync.dma_start(out=t_[:], in_=i_[:])

                xs1_shard = dram.tile([shard, h], fp16)
                xs1_full = dram.tile([xs1_rows, h], fp16)
                xs2_shard = dram.tile([G * 128, h], fp16)
                xs2_full = dram.tile([xs2_rows, h], fp16)

                # ---- stage A: xs1 tiles, staged + AllGather'd per chunk
                xsb = big.tile([128, ntile_nat * 128], fp16, name="xsb",
                               tag="big_a")
                row_base = 0
                for q, (lo, hi) in enumerate(ch_a):
                    for j in range(lo, hi):
                        lhsT = st.tile([128, 128], fp32, name="xTt",
                                       tag="lhsT", bufs=4)
                        nc.sync.dma_start(
                            out=lhsT[:], in_=xT_in[:, j * 128:(j + 1) * 128])
                        pxs = ps.tile([128, h], fp32, name="pxs", tag="pxs",
                                      bufs=2)
                        nc.tensor.matmul(pxs[:], lhsT[:], w1_t[:], start=True,
                                         stop=True)
                        nc.vector.tensor_scalar(
                            out=xsb[:, j * 128:(j + 1) * 128], in0=pxs[:],
                            scalar1=dn_t[:, j:j + 1], scalar2=None,
                            op0=mybir.AluOpType.mult)
                    rows = hi - lo
                    dest = bass.AP(xs1_shard[:].tensor, lo * 128 * h,
                                   [[h, 128], [128 * h, rows], [1, h]])
                    nc.sync.dma_start(out=dest, in_=xsb[:].rearrange(
                        "p (j f) -> p j f", f=h)[:, lo:hi, :])
                    nc.gpsimd.collective_compute(
                        "AllGather", mybir.AluOpType.bypass,
                        replica_groups=[list(range(nc_))],
                        ins=[xs1_shard[lo * 128:hi * 128, :].opt()],
                        outs=[xs1_full[row_base:row_base + nc_ * rows * 128,
                                       :].opt()],
                    )
                    row_base += nc_ * rows * 128

                # ---- stage C (layer 1) with interleaved stage D (xs2 + AG2)
                hT = big.tile([128, G * 128], fp32, name="hT", tag="big_b")
                xs2b = big.tile([128, G * 128], fp16, name="xs2b", tag="big_c")
                g_meta = {}
                acc = 0
                for q, (lo, hi) in enumerate(ch_d):
                    for g in range(lo, hi):
                        g_meta[g] = (q, lo, hi, acc)
                    acc += nc_ * (hi - lo) * 128

                def post_group(g):
                    pxs = ps.tile([128, h], fp32, name="pxs2", tag="pxs",
                                  bufs=2)
                    nc.tensor.matmul(pxs[:], hT[:, g * 128:(g + 1) * 128],
                                     w2_t[:], start=True, stop=True)
                    nc.vector.tensor_scalar(
                        out=xs2b[:, g * 128:(g + 1) * 128], in0=pxs[:],
                        scalar1=dp_t[:, g:g + 1], scalar2=None,
                        op0=mybir.AluOpType.mult)
                    q, lo, hi, dbase = g_meta[g]
                    if g == hi - 1:
                        rows = hi - lo
                        dest = bass.AP(xs2_shard[:].tensor, lo * 128 * h,
                                       [[h, 128], [128 * h, rows], [1, h]])
                        nc.sync.dma_start(out=dest, in_=xs2b[:].rearrange(
                            "p (j f) -> p j f", f=h)[:, lo:hi, :])
                        nc.gpsimd.collective_compute(
                            "AllGather", mybir.AluOpType.bypass,
                            replica_groups=[list(range(nc_))],
                            ins=[xs2_shard[lo * 128:hi * 128, :].opt()],
                            outs=[xs2_full[dbase:dbase + nc_ * rows * 128,
                                           :].opt()],
                        )

                edge_layer(src1_t, xs1_full[:], s1_t, t1_t, hT, post_group)
                nc.sync.dma_start(out=out_xs2[:], in_=xs2_full[:])
            else:
                wf_t = res.tile([h, 2], fp32)
                bf_t = res.tile([128, 2], fp32)
                s2_t = res.tile([h, 1], fp32)
                t2_t = res.tile([h, 1], fp32)
                src2_t = res.tile([128, ntiles], i32)
                for t_, i_ in ((wf_t, wf_in), (bf_t, bf_in), (s2_t, s2_in),
                               (t2_t, t2_in), (src2_t, src2_in)):
                    nc.sync.dma_start(out=t_[:], in_=i_[:])

                h2T = big.tile([128, G * 128], fp32, name="h2T", tag="big_b")
                edge_layer(src2_t, xs2_in[:], s2_t, t2_t, h2T)

                lg = res.tile([128, 2 * G], fp32)
                for j in range(G):
                    plg = ps.tile([128, 2], fp32, name="plg", tag="plg", bufs=2)
                    nc.tensor.matmul(plg[:], h2T[:, j * 128:(j + 1) * 128],
                                     wf_t[:], start=True, stop=True)
                    nc.vector.tensor_add(out=lg[:, 2 * j:2 * j + 2],
                                         in0=plg[:], in1=bf_t[:])

                def strided(base, start):
                    a = base[:]
                    return bass.AP(a.tensor, a.offset + start,
                                   [a.ap[0], [2, G]])

                z0, z1 = strided(lg, 0), strided(lg, 1)
                mx = res.tile([128, G], fp32)
                nc.vector.tensor_tensor(out=mx[:], in0=z0, in1=z1,
                                        op=mybir.AluOpType.max)
                sm0 = res.tile([128, G], fp32)
                sm1 = res.tile([128, G], fp32)
                nc.vector.tensor_sub(out=sm0[:], in0=z0, in1=mx[:])
                nc.vector.tensor_sub(out=sm1[:], in0=z1, in1=mx[:])
                e0 = res.tile([128, G], fp32)
                e1 = res.tile([128, G], fp32)
                nc.scalar.activation(out=e0[:], in_=sm0[:], func=AF.Exp)
                nc.scalar.activation(out=e1[:], in_=sm1[:], func=AF.Exp)
                se = res.tile([128, G], fp32)
                nc.vector.tensor_add(out=se[:], in0=e0[:], in1=e1[:])
                ls = res.tile([128, G], fp32)
                nc.scalar.activation(out=ls[:], in_=se[:], func=AF.Ln)
                nc.vector.tensor_sub(out=sm0[:], in0=sm0[:], in1=ls[:])
                nc.vector.tensor_sub(out=sm1[:], in0=sm1[:], in1=ls[:])
                lpo = res.tile([128, 2 * G], fp32)
                nc.vector.tensor_copy(out=strided(lpo, 0), in_=sm0[:])
                nc.vector.tensor_copy(out=strided(lpo, 1), in_=sm1[:])
                nc.sync.dma_start(out=out_lp[:], in_=lpo[:])

    nc.compile()
    return nc


# ---------------------------------------------------------------- main entry
def _run(x, edge_index, game_indices,
         W1, b1, g1, be1, m1, v1, W2, b2, g2, be2, m2, v2, Wf, bf,
         trace=False, cfg=None):
    from concourse import bass_utils

    if cfg is None:
        cfg = dict(N=N, NPAD=NPAD, SHARD=SHARD, NC=NC, GROUP_EDGES=GROUP_EDGES,
                   GROUP_TILES=GROUP_TILES, H=H, F_IN=F_IN, NCHUNK=NCHUNK)
    cfg.setdefault("NCHUNK", NCHUNK)

    x = np.asarray(x, dtype=np.float32)
    key = ("prep", x.shape, int(np.asarray(edge_index)[0, 0]),
           int(np.asarray(edge_index).sum() % (1 << 31)))
    if key in _CACHE:
        per_core, meta, G, ntiles = _CACHE[key]
    else:
        per_core, meta, G, ntiles = _prepare(x, np.asarray(edge_index), cfg)
        _CACHE.clear()
        _CACHE[key] = (per_core, meta, G, ntiles)

    bkey = ("bass", G, ntiles)
    if bkey in _CACHE:
        nc_a, nc_b = _CACHE[bkey]
    else:
        nc_a = _build(cfg, G, ntiles, "a", meta)
        nc_b = _build(cfg, G, ntiles, "b", meta)
        _CACHE[bkey] = (nc_a, nc_b)

    s1, t1 = _fold_bn(np.asarray(g1), np.asarray(be1), np.asarray(m1),
                      np.asarray(v1), np.asarray(b1))
    s2, t2 = _fold_bn(np.asarray(g2), np.asarray(be2), np.asarray(m2),
                      np.asarray(v2), np.asarray(b2))
    iota = np.broadcast_to(np.arange(128, dtype=np.float16),
                           (128, 128)).copy()
    bf_rep = np.broadcast_to(np.asarray(bf, dtype=np.float32), (128, 2)).copy()

    ncores = cfg["NC"]
    in_maps_a = []
    for c in range(ncores):
        pc = per_core[c]
        in_maps_a.append(dict(
            xT=pc["xT"], W1=np.asarray(W1, np.float32),
            W2=np.asarray(W2, np.float32), s1=s1, t1=t1, iota=iota,
            src1=pc["src1"], dloc=pc["dloc"], dinv_dst=pc["dinv_dst"],
            dinv_nat=pc["dinv_nat"], dinv_padlay=pc["dinv_padlay"],
        ))
    res_a = bass_utils.run_bass_kernel_spmd(
        nc_a, in_maps_a, core_ids=list(range(ncores)), trace=trace)

    in_maps_b = []
    for c in range(ncores):
        pc = per_core[c]
        in_maps_b.append(dict(
            xs2_full_in=res_a.results[c]["xs2_full_out"],
            Wf=np.asarray(Wf, np.float32), bf_rep=bf_rep, s2=s2, t2=t2,
            iota=iota, src2=pc["src2"], dloc=pc["dloc"],
            dinv_dst=pc["dinv_dst"],
        ))
    res_b = bass_utils.run_bass_kernel_spmd(
        nc_b, in_maps_b, core_ids=list(range(ncores)), trace=trace)

    class _Res:
        pass

    res = _Res()
    res.results = res_b.results
    res.exec_time_ns = ((res_a.exec_time_ns or 0) + (res_b.exec_time_ns or 0)) \
        if (res_a.exec_time_ns or res_b.exec_time_ns) else None
    res.parts = (res_a, res_b)

    gi = np.asarray(game_indices, dtype=np.int64)
    cji = meta["pad_cji"][gi]
    lp = np.stack([res_b.results[c]["logp"] for c in range(ncores)])
    out = np.empty((gi.shape[0], 2), dtype=np.float32)
    out[:, 0] = lp[cji[:, 0], cji[:, 2], 2 * cji[:, 1]]
    out[:, 1] = lp[cji[:, 0], cji[:, 2], 2 * cji[:, 1] + 1]
    return out, res


def kernel(**inputs):
    out, _ = _run(**inputs)
    return out


def kernel_profiled(**inputs):
    out, res = _run(**inputs, trace=True)
    return out, res



# revision 6
# speedup vs baseline: 1.3135x; 1.3135x over previous
"""Trainium2 Bass kernel for a 2-layer GCN (EnhancedHockeyGNN).

Strategy (8 NeuronCores, SPMD, two NEFF launches):
  - Nodes sharded row-wise across cores (dst ownership); weights replicated.
  - Per layer: xs = (x @ W) * dinv computed on the owning core and staged to
    DRAM as an fp16 message table, AllGather'd (in 4 overlapped chunks) into
    every core's DRAM.
  - Edges (incl. explicit self-loops) are sharded by dst owner and bin-packed
    into groups of <=128 dst nodes / <=2048 edges. Per group ONE multi-row
    indirect DMA (offset AP [128, 16]) fetches all 2048 message rows; a pure
    0/1 one-hot (is_equal on the vector engine) turns the segment-sum into a
    PSUM-accumulated fp16 matmul chain producing feature-major aggregates.
    The dinv[dst] normalization is applied post-PSUM as a per-column scale
    against a host-precomputed replicated-row table, then BN+ReLU folds into
    one scalar-engine activation.
  - Layer-2's message table is produced per-group and AllGather'd inside part
    A directly into its ExternalOutput, handed to part B as a plain input.
  - Readout computes log-softmax for every node; the host selects the
    requested game_indices rows (pure index routing).
"""
import math

import numpy as np

# ---------------------------------------------------------------- constants
N = 100000
F_IN = 128
H = 128
NC = 8
SHARD = 12544            # multiple of 128; 8 * 12544 = 100352 >= N
NPAD = NC * SHARD
GROUP_EDGES = 2048       # edges per group (16 tiles of 128)
GROUP_TILES = GROUP_EDGES // 128
GROUP_DSTS = 128         # max dst nodes per group (PSUM partition dim)
NCHUNK = 4               # AllGather overlap chunks
EPS = 1e-5

_CACHE = {}


def _chunks(n, k):
    """Split n items into k nearly-equal contiguous chunks: list of (lo, hi)."""
    k = min(k, n)
    base = n // k
    rem = n % k
    out = []
    lo = 0
    for i in range(k):
        hi = lo + base + (1 if i < rem else 0)
        out.append((lo, hi))
        lo = hi
    return out


# ---------------------------------------------------------------- host prep
def _bin_pack(counts, G):
    order = np.argsort(-counts, kind="stable")
    bin_edges = np.zeros(G, dtype=np.int64)
    bin_nodes = np.zeros(G, dtype=np.int64)
    group_of = np.full(counts.shape[0], -1, dtype=np.int32)
    pos_in_group = np.full(counts.shape[0], -1, dtype=np.int32)
    for d in order:
        c = counts[d]
        placed = False
        for b in range(G):
            if bin_edges[b] + c <= GROUP_EDGES and bin_nodes[b] < GROUP_DSTS:
                group_of[d] = b
                pos_in_group[d] = bin_nodes[b]
                bin_edges[b] += c
                bin_nodes[b] += 1
                placed = True
                break
        if not placed:
            return None
    return group_of, pos_in_group


def _prepare(x, edge_index, cfg):
    n, npad, shard, nc = cfg["N"], cfg["NPAD"], cfg["SHARD"], cfg["NC"]
    ge, gt = cfg["GROUP_EDGES"], cfg["GROUP_TILES"]

    src = np.asarray(edge_index[0], dtype=np.int64)
    dst = np.asarray(edge_index[1], dtype=np.int64)
    deg = np.bincount(dst, minlength=n).astype(np.float64) + 1.0
    dinv = (1.0 / np.sqrt(deg)).astype(np.float32)
    dinv_pad_full = np.ones(npad, dtype=np.float32)
    dinv_pad_full[:n] = dinv

    sall = np.concatenate([src, np.arange(n, dtype=np.int64)])
    dall = np.concatenate([dst, np.arange(n, dtype=np.int64)])
    owner = dall // shard

    Es = [int((owner == c).sum()) for c in range(nc)]
    G = max(int(math.ceil(e / ge)) for e in Es)
    while True:
        packs = []
        ok = True
        for c in range(nc):
            m = owner == c
            d0 = (dall[m] - c * shard).astype(np.int64)
            counts = np.bincount(d0, minlength=shard)
            r = _bin_pack(counts, G)
            if r is None:
                ok = False
                break
            packs.append((r[0], r[1], d0, sall[m]))
        if ok:
            break
        G += 1

    ntiles = G * gt
    ntile_nat = shard // 128
    nchunk = cfg["NCHUNK"]

    # ----- chunk-major table layouts (AllGather per chunk)
    ch_a = _chunks(ntile_nat, nchunk)
    chunk_base_a = []
    acc = 0
    for lo, hi in ch_a:
        chunk_base_a.append(acc)
        acc += nc * (hi - lo) * 128
    tile_q = np.zeros(ntile_nat, dtype=np.int64)
    for q, (lo, hi) in enumerate(ch_a):
        tile_q[lo:hi] = q
    nodes = np.arange(npad, dtype=np.int64)
    c_of = nodes // shard
    loc = nodes % shard
    j_of = loc // 128
    p_of = loc % 128
    q_of = tile_q[j_of]
    rows_q = np.array([hi - lo for lo, hi in ch_a], dtype=np.int64)[q_of] * 128
    lo_q = np.array([lo for lo, hi in ch_a], dtype=np.int64)[q_of]
    base_q = np.array(chunk_base_a, dtype=np.int64)[q_of]
    xs1_row = base_q + c_of * rows_q + (j_of - lo_q) * 128 + p_of

    ch_d = _chunks(G, nchunk)
    chunk_base_d = []
    acc = 0
    for lo, hi in ch_d:
        chunk_base_d.append(acc)
        acc += nc * (hi - lo) * 128
    g_q = np.zeros(G, dtype=np.int64)
    for q, (lo, hi) in enumerate(ch_d):
        g_q[lo:hi] = q

    padded_row = np.zeros(npad, dtype=np.int64)      # node -> xs2 row
    pad_cji = np.zeros((npad, 3), dtype=np.int64)    # node -> (core, group, pos)
    for c in range(nc):
        group_of, pos, _, _ = packs[c]
        g64 = group_of.astype(np.int64)
        p64 = pos.astype(np.int64)
        q = g_q[g64]
        lo = np.array([l for l, _ in ch_d], dtype=np.int64)[q]
        hi = np.array([h_ for _, h_ in ch_d], dtype=np.int64)[q]
        base = np.array(chunk_base_d, dtype=np.int64)[q]
        rows = c * (hi - lo) * 128 + (g64 - lo) * 128 + p64 + base
        padded_row[c * shard:(c + 1) * shard] = rows
        pad_cji[c * shard:(c + 1) * shard, 0] = c
        pad_cji[c * shard:(c + 1) * shard, 1] = g64
        pad_cji[c * shard:(c + 1) * shard, 2] = p64

    per_core = []
    for c in range(nc):
        group_of, pos, d0, s_nodes = packs[c]
        g_of_edge = group_of[d0]
        order = np.argsort(g_of_edge, kind="stable")
        d0o, so, go = d0[order], s_nodes[order], g_of_edge[order]
        src1 = np.zeros((128, ntiles), dtype=np.int32)
        src2 = np.zeros((128, ntiles), dtype=np.int32)
        dloc = np.full((128, ntiles), 300.0, dtype=np.float16)
        gstart = np.searchsorted(go, np.arange(G))
        gend = np.searchsorted(go, np.arange(G) + 1)
        for g in range(G):
            a, b = int(gstart[g]), int(gend[g])
            k = b - a
            assert k <= ge
            sl_s = so[a:b]
            sl_d = d0o[a:b]
            t = np.arange(k) // 128
            p = np.arange(k) % 128
            cols = g * gt + t
            src1[p, cols] = xs1_row[sl_s]
            src2[p, cols] = padded_row[sl_s]
            dloc[p, cols] = pos[sl_d]
        jj = np.arange(shard)
        dinv_nat = dinv_pad_full[c * shard + jj].reshape(shard // 128, 128).T.copy()
        xs_shape = np.zeros((shard, x.shape[1]), dtype=np.float32)
        lo, hi = c * shard, min((c + 1) * shard, n)
        xs_shape[: hi - lo] = x[lo:hi]
        xT = np.ascontiguousarray(xs_shape.T).astype(np.float16)
        inv_nodes = np.full(G * 128, -1, dtype=np.int64)
        inv_nodes[group_of.astype(np.int64) * 128 + pos.astype(np.int64)] = \
            np.arange(shard)
        valid = inv_nodes >= 0
        vals = np.zeros(G * 128, dtype=np.float32)
        vals[valid] = dinv_pad_full[c * shard + inv_nodes[valid]]
        dinv_padlay = vals.reshape(G, 128).T.copy()
        ddrow = np.broadcast_to(vals.astype(np.float16)[None, :],
                                (128, G * 128)).copy()
        per_core.append(dict(src1=src1, src2=src2, dloc=dloc,
                             dinv_nat=dinv_nat, dinv_padlay=dinv_padlay,
                             ddrow=ddrow, xT=xT))
    meta = dict(ch_a=ch_a, ch_d=ch_d, pad_cji=pad_cji)
    return per_core, meta, G, ntiles


def _fold_bn(gamma, beta, mean, var, b):
    s = (gamma / np.sqrt(var + EPS)).astype(np.float32)
    t = ((b - mean) * s + beta).astype(np.float32)
    return s.reshape(H, 1), t.reshape(H, 1)


# ---------------------------------------------------------------- bass build
def _build(cfg, G, ntiles, part, meta):
    import concourse.bacc as bacc
    import concourse.bass as bass
    import concourse.mybir as mybir
    import concourse.tile as tile

    fp32 = mybir.dt.float32
    fp16 = mybir.dt.float16
    i32 = mybir.dt.int32
    AF = mybir.ActivationFunctionType

    nc_ = cfg["NC"]
    shard = cfg["SHARD"]
    gt = cfg["GROUP_TILES"]
    ntile_nat = shard // 128
    h = cfg["H"]
    fin = cfg["F_IN"]
    ch_a = meta["ch_a"]
    ch_d = meta["ch_d"]
    xs1_rows = nc_ * ntile_nat * 128
    xs2_rows = nc_ * G * 128

    nc = bacc.Bacc(None, target_bir_lowering=False, debug=False, num_devices=nc_)

    iota_in = nc.dram_tensor("iota", [128, gt * 128], fp16, kind="ExternalInput")
    dloc_in = nc.dram_tensor("dloc", [128, ntiles], fp16, kind="ExternalInput")
    ddrow_in = nc.dram_tensor("ddrow", [128, G * 128], fp16,
                              kind="ExternalInput")

    if part == "a":
        xT_in = nc.dram_tensor("xT", [fin, shard], fp16, kind="ExternalInput")
        w1_in = nc.dram_tensor("W1", [fin, h], fp16, kind="ExternalInput")
        w2_in = nc.dram_tensor("W2", [h, h], fp16, kind="ExternalInput")
        s1_in = nc.dram_tensor("s1", [h, 1], fp32, kind="ExternalInput")
        t1_in = nc.dram_tensor("t1", [h, 1], fp32, kind="ExternalInput")
        src1_in = nc.dram_tensor("src1", [128, ntiles], i32, kind="ExternalInput")
        dn_in = nc.dram_tensor("dinv_nat", [128, ntile_nat], fp32,
                               kind="ExternalInput")
        dp_in = nc.dram_tensor("dinv_padlay", [128, G], fp32,
                               kind="ExternalInput")
        out_xs2 = nc.dram_tensor("xs2_full_out", [xs2_rows, h], fp16,
                                 kind="ExternalOutput")
    else:
        xs2_in = nc.dram_tensor("xs2_full_in", [xs2_rows, h], fp16,
                                kind="ExternalInput")
        wf_in = nc.dram_tensor("Wf", [h, 2], fp16, kind="ExternalInput")
        bf_in = nc.dram_tensor("bf_rep", [128, 2], fp32, kind="ExternalInput")
        s2_in = nc.dram_tensor("s2", [h, 1], fp32, kind="ExternalInput")
        t2_in = nc.dram_tensor("t2", [h, 1], fp32, kind="ExternalInput")
        src2_in = nc.dram_tensor("src2", [128, ntiles], i32, kind="ExternalInput")
        out_lp = nc.dram_tensor("logp", [128, 2 * G], fp32, kind="ExternalOutput")

    with tile.TileContext(nc) as tc:
        with (
            tc.tile_pool(name="res", bufs=1) as res,
            tc.tile_pool(name="big", bufs=1) as big,
            tc.tile_pool(name="stream", bufs=1) as st,
            tc.tile_pool(name="ps", bufs=1, space="PSUM") as ps,
            tc.tile_pool(name="dram", bufs=1, space="DRAM") as dram,
        ):
            iota_t = res.tile([128, gt, 128], fp16)
            dloc_t = res.tile([128, ntiles], fp16)
            ddrow_t = res.tile([128, G * 128], fp16)
            nc.sync.dma_start(out=iota_t[:],
                              in_=iota_in[:].rearrange("p (k d) -> p k d", d=128))
            nc.sync.dma_start(out=dloc_t[:], in_=dloc_in[:])
            nc.sync.dma_start(out=ddrow_t[:], in_=ddrow_in[:])

            def edge_layer(src_t, xs_full_ap, s_t, t_t, post_group):
                for g in range(G):
                    oh = st.tile([128, gt, 128], fp16, name="oh", tag="oh",
                                 bufs=3)
                    nc.vector.tensor_tensor(
                        out=oh[:],
                        in0=dloc_t[:, g * gt:(g + 1) * gt].to_broadcast(
                            [128, gt, 128]),
                        in1=iota_t[:],
                        op=mybir.AluOpType.is_equal,
                    )
                    msg = st.tile([128, gt, h], fp16, name="msg", tag="msg",
                                  bufs=3)
                    for t in range(gt):
                        k = g * gt + t
                        nc.gpsimd.indirect_dma_start(
                            out=msg[:, t, :],
                            out_offset=None,
                            in_=xs_full_ap,
                            in_offset=bass.IndirectOffsetOnAxis(
                                ap=src_t[:, k:k + 1], axis=0),
                        )
                    pg = ps.tile([h, 128], fp32, name="pg", tag="pg", bufs=4)
                    for t in range(gt):
                        nc.tensor.matmul(pg[:], msg[:, t, :], oh[:, t, :],
                                         start=(t == 0), stop=(t == gt - 1))
                    tmp = st.tile([h, 128], fp32, name="tmp", tag="tmp",
                                  bufs=4)
                    nc.vector.tensor_tensor(
                        out=tmp[:], in0=pg[:],
                        in1=ddrow_t[:, g * 128:(g + 1) * 128],
                        op=mybir.AluOpType.mult,
                    )
                    hblk = st.tile([h, 128], fp16, name="hblk", tag="hblk",
                                   bufs=4)
                    nc.scalar.activation(
                        out=hblk[:], in_=tmp[:],
                        func=AF.Relu, bias=t_t[:], scale=s_t[:],
                    )
                    post_group(g, hblk)

            if part == "a":
                w1_t = res.tile([fin, h], fp16)
                w2_t = res.tile([h, h], fp16)
                s1_t = res.tile([h, 1], fp32)
                t1_t = res.tile([h, 1], fp32)
                src1_t = res.tile([128, ntiles], i32)
                dn_t = res.tile([128, ntile_nat], fp32)
                dp_t = res.tile([128, G], fp32)
                for t_, i_ in ((w1_t, w1_in), (w2_t, w2_in), (s1_t, s1_in),
                               (t1_t, t1_in), (src1_t, src1_in), (dn_t, dn_in),
                               (dp_t, dp_in)):
                    nc.sync.dma_start(out=t_[:], in_=i_[:])

                xs1_shard = dram.tile([shard, h], fp16)
                xs1_full = dram.tile([xs1_rows, h], fp16)

                # ---- stage A: xs1 tiles, staged + AllGather'd per chunk
                xsb = big.tile([128, ntile_nat * 128], fp16, name="xsb",
                               tag="big_a")
                row_base = 0
                for q, (lo, hi) in enumerate(ch_a):
                    for j in range(lo, hi):
                        lhsT = st.tile([128, 128], fp16, name="xTt",
                                       tag="lhsT", bufs=4)
                        nc.sync.dma_start(
                            out=lhsT[:], in_=xT_in[:, j * 128:(j + 1) * 128])
                        pxs = ps.tile([128, h], fp32, name="pxs", tag="pxs",
                                      bufs=2)
                        nc.tensor.matmul(pxs[:], lhsT[:], w1_t[:], start=True,
                                         stop=True)
                        nc.vector.tensor_scalar(
                            out=xsb[:, j * 128:(j + 1) * 128], in0=pxs[:],
                            scalar1=dn_t[:, j:j + 1], scalar2=None,
                            op0=mybir.AluOpType.mult)
                    rows = hi - lo
                    dest = bass.AP(xs1_shard[:].tensor, lo * 128 * h,
                                   [[h, 128], [128 * h, rows], [1, h]])
                    nc.sync.dma_start(out=dest, in_=xsb[:].rearrange(
                        "p (j f) -> p j f", f=h)[:, lo:hi, :])
                    nc.gpsimd.collective_compute(
                        "AllGather", mybir.AluOpType.bypass,
                        replica_groups=[list(range(nc_))],
                        ins=[xs1_shard[lo * 128:hi * 128, :].opt()],
                        outs=[xs1_full[row_base:row_base + nc_ * rows * 128,
                                       :].opt()],
                    )
                    row_base += nc_ * rows * 128

                # ---- layer 1 with interleaved xs2 production + AG2
                xs2_shard = dram.tile([G * 128, h], fp16)
                xs2_full = dram.tile([xs2_rows, h], fp16)
                xs2b = big.tile([128, G * 128], fp16, name="xs2b", tag="big_c")
                g_meta = {}
                acc = 0
                for q, (lo, hi) in enumerate(ch_d):
                    for g in range(lo, hi):
                        g_meta[g] = (q, lo, hi, acc)
                    acc += nc_ * (hi - lo) * 128

                def post_group(g, hblk):
                    pxs = ps.tile([128, h], fp32, name="pxs2", tag="pxs",
                                  bufs=2)
                    nc.tensor.matmul(pxs[:], hblk[:], w2_t[:], start=True,
                                     stop=True)
                    nc.vector.tensor_scalar(
                        out=xs2b[:, g * 128:(g + 1) * 128], in0=pxs[:],
                        scalar1=dp_t[:, g:g + 1], scalar2=None,
                        op0=mybir.AluOpType.mult)
                    q, lo, hi, dbase = g_meta[g]
                    if g == hi - 1:
                        rows = hi - lo
                        dest = bass.AP(xs2_shard[:].tensor, lo * 128 * h,
                                       [[h, 128], [128 * h, rows], [1, h]])
                        nc.sync.dma_start(out=dest, in_=xs2b[:].rearrange(
                            "p (j f) -> p j f", f=h)[:, lo:hi, :])
                        nc.gpsimd.collective_compute(
                            "AllGather", mybir.AluOpType.bypass,
                            replica_groups=[list(range(nc_))],
                            ins=[xs2_shard[lo * 128:hi * 128, :].opt()],
                            outs=[xs2_full[dbase:dbase + nc_ * rows * 128,
                                           :].opt()],
                        )
                        nc.sync.dma_start(
                            out=out_xs2[dbase:dbase + nc_ * rows * 128, :],
                            in_=xs2_full[dbase:dbase + nc_ * rows * 128, :])

                edge_layer(src1_t, xs1_full[:], s1_t, t1_t, post_group)
            else:
                wf_t = res.tile([h, 2], fp16)
                bf_t = res.tile([128, 2], fp32)
                s2_t = res.tile([h, 1], fp32)
                t2_t = res.tile([h, 1], fp32)
                src2_t = res.tile([128, ntiles], i32)
                for t_, i_ in ((wf_t, wf_in), (bf_t, bf_in), (s2_t, s2_in),
                               (t2_t, t2_in), (src2_t, src2_in)):
                    nc.sync.dma_start(out=t_[:], in_=i_[:])

                lg = res.tile([128, 2 * G], fp32)

                def post_group_b(g, hblk):
                    plg = ps.tile([128, 2], fp32, name="plg", tag="plg",
                                  bufs=2)
                    nc.tensor.matmul(plg[:], hblk[:], wf_t[:], start=True,
                                     stop=True)
                    nc.vector.tensor_add(out=lg[:, 2 * g:2 * g + 2],
                                         in0=plg[:], in1=bf_t[:])

                edge_layer(src2_t, xs2_in[:], s2_t, t2_t, post_group_b)

                def strided(base, start):
                    a = base[:]
                    return bass.AP(a.tensor, a.offset + start,
                                   [a.ap[0], [2, G]])

                z0, z1 = strided(lg, 0), strided(lg, 1)
                mx = res.tile([128, G], fp32)
                nc.vector.tensor_tensor(out=mx[:], in0=z0, in1=z1,
                                        op=mybir.AluOpType.max)
                sm0 = res.tile([128, G], fp32)
                sm1 = res.tile([128, G], fp32)
                nc.vector.tensor_sub(out=sm0[:], in0=z0, in1=mx[:])
                nc.vector.tensor_sub(out=sm1[:], in0=z1, in1=mx[:])
                e0 = res.tile([128, G], fp32)
                e1 = res.tile([128, G], fp32)
                nc.scalar.activation(out=e0[:], in_=sm0[:], func=AF.Exp)
                nc.scalar.activation(out=e1[:], in_=sm1[:], func=AF.Exp)
                se = res.tile([128, G], fp32)
                nc.vector.tensor_add(out=se[:], in0=e0[:], in1=e1[:])
                ls = res.tile([128, G], fp32)
                nc.scalar.activation(out=ls[:], in_=se[:], func=AF.Ln)
                nc.vector.tensor_sub(out=sm0[:], in0=sm0[:], in1=ls[:])
                nc.vector.tensor_sub(out=sm1[:], in0=sm1[:], in1=ls[:])
                lpo = res.tile([128, 2 * G], fp32)
                nc.vector.tensor_copy(out=strided(lpo, 0), in_=sm0[:])
                nc.vector.tensor_copy(out=strided(lpo, 1), in_=sm1[:])
                nc.sync.dma_start(out=out_lp[:], in_=lpo[:])

    nc.compile()
    return nc


# ---------------------------------------------------------------- main entry
def _run(x, edge_index, game_indices,
         W1, b1, g1, be1, m1, v1, W2, b2, g2, be2, m2, v2, Wf, bf,
         trace=False, cfg=None):
    from concourse import bass_utils

    if cfg is None:
        cfg = dict(N=N, NPAD=NPAD, SHARD=SHARD, NC=NC, GROUP_EDGES=GROUP_EDGES,
                   GROUP_TILES=GROUP_TILES, H=H, F_IN=F_IN, NCHUNK=NCHUNK)
    cfg.setdefault("NCHUNK", NCHUNK)

    x = np.asarray(x, dtype=np.float32)
    key = ("prep", x.shape, int(np.asarray(edge_index)[0, 0]),
           int(np.asarray(edge_index).sum() % (1 << 31)))
    if key in _CACHE:
        per_core, meta, G, ntiles = _CACHE[key]
    else:
        per_core, meta, G, ntiles = _prepare(x, np.asarray(edge_index), cfg)
        _CACHE.clear()
        _CACHE[key] = (per_core, meta, G, ntiles)

    bkey = ("bass", G, ntiles)
    if bkey in _CACHE:
        nc_a, nc_b = _CACHE[bkey]
    else:
        nc_a = _build(cfg, G, ntiles, "a", meta)
        nc_b = _build(cfg, G, ntiles, "b", meta)
        _CACHE[bkey] = (nc_a, nc_b)

    s1, t1 = _fold_bn(np.asarray(g1), np.asarray(be1), np.asarray(m1),
                      np.asarray(v1), np.asarray(b1))
    s2, t2 = _fold_bn(np.asarray(g2), np.asarray(be2), np.asarray(m2),
                      np.asarray(v2), np.asarray(b2))
    gtl = cfg["GROUP_TILES"]
    iota = np.tile(np.arange(128, dtype=np.float16), (128, gtl)).copy()
    bf_rep = np.broadcast_to(np.asarray(bf, dtype=np.float32), (128, 2)).copy()

    ncores = cfg["NC"]
    in_maps_a = []
    for c in range(ncores):
        pc = per_core[c]
        in_maps_a.append(dict(
            xT=pc["xT"], W1=np.asarray(W1, np.float16),
            W2=np.asarray(W2, np.float16), s1=s1, t1=t1, iota=iota,
            src1=pc["src1"], dloc=pc["dloc"], ddrow=pc["ddrow"],
            dinv_nat=pc["dinv_nat"], dinv_padlay=pc["dinv_padlay"],
        ))
    res_a = bass_utils.run_bass_kernel_spmd(
        nc_a, in_maps_a, core_ids=list(range(ncores)), trace=trace)

    in_maps_b = []
    for c in range(ncores):
        pc = per_core[c]
        in_maps_b.append(dict(
            xs2_full_in=res_a.results[c]["xs2_full_out"],
            Wf=np.asarray(Wf, np.float16), bf_rep=bf_rep, s2=s2, t2=t2,
            iota=iota, src2=pc["src2"], dloc=pc["dloc"], ddrow=pc["ddrow"],
        ))
    res_b = bass_utils.run_bass_kernel_spmd(
        nc_b, in_maps_b, core_ids=list(range(ncores)), trace=trace)

    class _Res:
        pass

    res = _Res()
    res.results = res_b.results
    res.exec_time_ns = ((res_a.exec_time_ns or 0) + (res_b.exec_time_ns or 0)) \
        if (res_a.exec_time_ns or res_b.exec_time_ns) else None
    res.parts = (res_a, res_b)

    gi = np.asarray(game_indices, dtype=np.int64)
    cji = meta["pad_cji"][gi]
    lp = np.stack([res_b.results[c]["logp"] for c in range(ncores)])
    out = np.empty((gi.shape[0], 2), dtype=np.float32)
    out[:, 0] = lp[cji[:, 0], cji[:, 2], 2 * cji[:, 1]]
    out[:, 1] = lp[cji[:, 0], cji[:, 2], 2 * cji[:, 1] + 1]
    return out, res


def kernel(**inputs):
    out, _ = _run(**inputs)
    return out


def kernel_profiled(**inputs):
    out, res = _run(**inputs, trace=True)
    return out, res


# revision 7
# speedup vs baseline: 2.9587x; 2.2526x over previous
"""Trainium2 Bass kernel for a 2-layer GCN (EnhancedHockeyGNN) — v3.

Changes vs v2:
  - Message tables are split into NCHUNK per-chunk DRAM tables (< 32768 rows
    each) so the gather can use the GPSIMD dma_gather custom instruction with
    int16 indices: ONE instruction per (cohort of COH groups, chunk) instead
    of one per 128-edge tile — ~10x less Pool-engine SWDGE time.
  - Edges of each group are bucketed by src chunk; tiles are per (group,
    chunk bucket), so layer-1 gathers of chunk q only wait on AllGather
    chunk q (overlaps the AG1 serial phase).
  - Layer-1 AllGather outputs are Internal addr_space="Shared" (single
    writer per chunk table) for the fast HBM-HBM collective path.
  - One-hot is_equal has the contiguous iota as in0 (double-pump port 0).
"""
import math

import numpy as np

# ---------------------------------------------------------------- constants
N = 100000
F_IN = 128
H = 128
NC = 8
SHARD = 12544            # multiple of 128; 8 * 12544 = 100352 >= N
NPAD = NC * SHARD
GROUP_EDGES = 2048       # edge budget per group
GROUP_DSTS = 128         # max dst nodes per group (PSUM partition dim)
NCHUNK = 4               # AllGather chunks == src buckets (int16 idx limit)
COH = 4                  # groups per gather cohort
EPS = 1e-5

_CACHE = {}


def _chunks(n, k):
    k = min(k, n)
    base, rem = n // k, n % k
    out, lo = [], 0
    for i in range(k):
        hi = lo + base + (1 if i < rem else 0)
        out.append((lo, hi))
        lo = hi
    return out


# ---------------------------------------------------------------- host prep
def _bin_pack(counts, G):
    order = np.argsort(-counts, kind="stable")
    bin_edges = np.zeros(G, dtype=np.int64)
    bin_nodes = np.zeros(G, dtype=np.int64)
    group_of = np.full(counts.shape[0], -1, dtype=np.int32)
    pos_in_group = np.full(counts.shape[0], -1, dtype=np.int32)
    for d in order:
        c = counts[d]
        placed = False
        for b in range(G):
            if bin_edges[b] + c <= GROUP_EDGES and bin_nodes[b] < GROUP_DSTS:
                group_of[d] = b
                pos_in_group[d] = bin_nodes[b]
                bin_edges[b] += c
                bin_nodes[b] += 1
                placed = True
                break
        if not placed:
            return None
    return group_of, pos_in_group


def _wrap_idx16(idx_flat):
    """[n] int16 -> [128, n//16] wrapped (i -> [i%16, i//16]) + replicated."""
    n = idx_flat.shape[0]
    assert n % 16 == 0
    w = idx_flat.reshape(n // 16, 16).T            # [16, cols]
    return np.tile(w, (8, 1)).copy()               # [128, cols]


def _prepare(x, edge_index, cfg):
    n, npad, shard, nc = cfg["N"], cfg["NPAD"], cfg["SHARD"], cfg["NC"]
    ge = cfg["GROUP_EDGES"]
    nchunk = cfg["NCHUNK"]
    coh = cfg["COH"]

    src = np.asarray(edge_index[0], dtype=np.int64)
    dst = np.asarray(edge_index[1], dtype=np.int64)
    deg = np.bincount(dst, minlength=n).astype(np.float64) + 1.0
    dinv = (1.0 / np.sqrt(deg)).astype(np.float32)
    dinv_pad_full = np.ones(npad, dtype=np.float32)
    dinv_pad_full[:n] = dinv

    sall = np.concatenate([src, np.arange(n, dtype=np.int64)])
    dall = np.concatenate([dst, np.arange(n, dtype=np.int64)])
    owner = dall // shard

    Es = [int((owner == c).sum()) for c in range(nc)]
    G = max(int(math.ceil(e / ge)) for e in Es)
    while True:
        packs = []
        ok = True
        for c in range(nc):
            m = owner == c
            d0 = (dall[m] - c * shard).astype(np.int64)
            counts = np.bincount(d0, minlength=shard)
            r = _bin_pack(counts, G)
            if r is None:
                ok = False
                break
            packs.append((r[0], r[1], d0, sall[m]))
        if ok:
            break
        G += 1

    ntile_nat = shard // 128

    # ----- chunk layouts
    ch_a = _chunks(ntile_nat, nchunk)          # layer-1 src buckets (nat tiles)
    ch_d = _chunks(G, nchunk)                  # layer-2 src buckets (groups)
    rows_a = [nc * (hi - lo) * 128 for lo, hi in ch_a]
    rows_d = [nc * (hi - lo) * 128 for lo, hi in ch_d]
    assert max(rows_a + rows_d) <= 32767 + 1

    nodes = np.arange(npad, dtype=np.int64)
    c_of = nodes // shard
    loc = nodes % shard
    j_of = loc // 128
    p_of = loc % 128
    tile_q = np.zeros(ntile_nat, dtype=np.int64)
    for q, (lo, hi) in enumerate(ch_a):
        tile_q[lo:hi] = q
    qa_of = tile_q[j_of]                        # layer-1 chunk of node
    lo_a = np.array([lo for lo, hi in ch_a], dtype=np.int64)[qa_of]
    nrows_a = np.array([hi - lo for lo, hi in ch_a], dtype=np.int64)[qa_of]
    row1_in_chunk = c_of * nrows_a * 128 + (j_of - lo_a) * 128 + p_of

    g_q = np.zeros(G, dtype=np.int64)
    for q, (lo, hi) in enumerate(ch_d):
        g_q[lo:hi] = q

    row2_in_chunk = np.zeros(npad, dtype=np.int64)
    qb_node = np.zeros(npad, dtype=np.int64)
    pad_cji = np.zeros((npad, 3), dtype=np.int64)
    for c in range(nc):
        group_of, pos, _, _ = packs[c]
        g64 = group_of.astype(np.int64)
        p64 = pos.astype(np.int64)
        q = g_q[g64]
        lo = np.array([l for l, _ in ch_d], dtype=np.int64)[q]
        hi = np.array([h_ for _, h_ in ch_d], dtype=np.int64)[q]
        rows = c * (hi - lo) * 128 + (g64 - lo) * 128 + p64
        row2_in_chunk[c * shard:(c + 1) * shard] = rows
        qb_node[c * shard:(c + 1) * shard] = q
        pad_cji[c * shard:(c + 1) * shard, 0] = c
        pad_cji[c * shard:(c + 1) * shard, 1] = g64
        pad_cji[c * shard:(c + 1) * shard, 2] = p64

    ncoh = (G + coh - 1) // coh

    def collect_layer(d0o, so, gstart, gend, layer):
        """Per (g, q) edge (row, dst) lists for one core/layer."""
        if layer == 1:
            q_of_edge = qa_of[so]
            row_of_edge = row1_in_chunk[so]
        else:
            q_of_edge = qb_node[so]
            row_of_edge = row2_in_chunk[so]
        edge_lists = {}
        for g in range(G):
            a, b = int(gstart[g]), int(gend[g])
            qe = q_of_edge[a:b]
            for q in range(nchunk):
                m = qe == q
                edge_lists[(g, q)] = (row_of_edge[a:b][m], d0o[a:b][m])
        return edge_lists

    def emit_layer(edge_lists, pos, T_gq):
        """Build tables for one core given the COMMON tile counts T_gq.

        call_meta[hcoh] is a list of (q, col_lo, ncols, ntiles, buf_off)
        sub-calls, each gathering <= NIDX_CAP rows.
        """
        cap_tiles = cfg.get("NIDX_CAP", 2048) // 128
        idx_cols = []
        call_meta = []
        msg_pos = [[] for _ in range(G)]
        dloc_cols = []
        oh_tiles = [int(T_gq[g].sum()) for g in range(G)]
        col_base = 0
        for hcoh in range(ncoh):
            gs = range(hcoh * coh, min((hcoh + 1) * coh, G))
            meta_h = []
            buf_off = 0
            for q in range(nchunk):
                tiles_q = 0
                idx_call = []
                for g in gs:
                    rows_e, _ = edge_lists[(g, q)]
                    T = int(T_gq[g, q])
                    assert rows_e.shape[0] <= T * 128
                    padded = np.zeros(T * 128, dtype=np.int16)
                    padded[: rows_e.shape[0]] = rows_e.astype(np.int16)
                    idx_call.append(padded)
                    for i in range(T):
                        msg_pos[g].append(buf_off + tiles_q + i)
                    tiles_q += T
                if tiles_q == 0:
                    continue
                flat = np.concatenate(idx_call)
                # split into sub-calls of <= cap_tiles tiles
                t0 = 0
                while t0 < tiles_q:
                    tpiece = min(cap_tiles, tiles_q - t0)
                    piece = flat[t0 * 128:(t0 + tpiece) * 128]
                    idx_cols.append(_wrap_idx16(piece))
                    ncols = piece.shape[0] // 16
                    meta_h.append((q, col_base, ncols, tpiece, buf_off + t0))
                    col_base += ncols
                    t0 += tpiece
                buf_off += tiles_q
            call_meta.append(meta_h)

        for g in range(G):
            for q in range(nchunk):
                _, d_e = edge_lists[(g, q)]
                T = int(T_gq[g, q])
                dl = np.full(T * 128, 300.0, dtype=np.float16)
                dl[: d_e.shape[0]] = pos[d_e]
                dloc_cols.append(dl.reshape(T, 128).T)   # [128, T]

        idx16 = np.concatenate(idx_cols, axis=1) if idx_cols else \
            np.zeros((128, 0), dtype=np.int16)
        dloc = np.concatenate(dloc_cols, axis=1)         # [128, T_total]
        return dict(idx16=idx16, dloc=dloc, call_meta=call_meta,
                    msg_pos=msg_pos, oh_tiles=oh_tiles,
                    T_total=dloc.shape[1])

    # pass 1: per-core edge lists; common (max) tile counts
    core_misc = []
    T1_gq = np.zeros((G, nchunk), dtype=np.int64)
    T2_gq = np.zeros((G, nchunk), dtype=np.int64)
    for c in range(nc):
        group_of, pos, d0, s_nodes = packs[c]
        g_of_edge = group_of[d0]
        order = np.argsort(g_of_edge, kind="stable")
        d0o, so = d0[order], s_nodes[order]
        go = g_of_edge[order]
        gstart = np.searchsorted(go, np.arange(G))
        gend = np.searchsorted(go, np.arange(G) + 1)
        el1 = collect_layer(d0o, so, gstart, gend, 1)
        el2 = collect_layer(d0o, so, gstart, gend, 2)
        for g in range(G):
            for q in range(nchunk):
                T1_gq[g, q] = max(T1_gq[g, q],
                                  (el1[(g, q)][0].shape[0] + 127) // 128)
                T2_gq[g, q] = max(T2_gq[g, q],
                                  (el2[(g, q)][0].shape[0] + 127) // 128)
        core_misc.append((el1, el2, pos, group_of))

    per_core = []
    for c in range(nc):
        el1, el2, pos, group_of = core_misc[c]
        L1 = emit_layer(el1, pos, T1_gq)
        L2 = emit_layer(el2, pos, T2_gq)

        jj = np.arange(shard)
        dinv_nat = dinv_pad_full[c * shard + jj].reshape(shard // 128, 128).T.copy()
        xs_shape = np.zeros((shard, x.shape[1]), dtype=np.float32)
        lo, hi = c * shard, min((c + 1) * shard, n)
        xs_shape[: hi - lo] = x[lo:hi]
        xT = np.ascontiguousarray(xs_shape.T).astype(np.float16)
        inv_nodes = np.full(G * 128, -1, dtype=np.int64)
        inv_nodes[group_of.astype(np.int64) * 128 + pos.astype(np.int64)] = \
            np.arange(shard)
        valid = inv_nodes >= 0
        vals = np.zeros(G * 128, dtype=np.float32)
        vals[valid] = dinv_pad_full[c * shard + inv_nodes[valid]]
        dinv_padlay = vals.reshape(G, 128).T.copy()
        ddrow = np.broadcast_to(vals.astype(np.float16)[None, :],
                                (128, G * 128)).copy()
        per_core.append(dict(L1=L1, L2=L2, dinv_nat=dinv_nat,
                             dinv_padlay=dinv_padlay, ddrow=ddrow, xT=xT))
    meta = dict(ch_a=ch_a, ch_d=ch_d, pad_cji=pad_cji,
                rows_a=rows_a, rows_d=rows_d)
    return per_core, meta, G


def _fold_bn(gamma, beta, mean, var, b):
    s = (gamma / np.sqrt(var + EPS)).astype(np.float32)
    t = ((b - mean) * s + beta).astype(np.float32)
    return s.reshape(H, 1), t.reshape(H, 1)


# ---------------------------------------------------------------- bass build
def _build(cfg, G, part, meta, L):
    import concourse.bacc as bacc
    import concourse.bass as bass
    import concourse.mybir as mybir
    import concourse.tile as tile

    fp32 = mybir.dt.float32
    fp16 = mybir.dt.float16
    i16 = mybir.dt.int16
    AF = mybir.ActivationFunctionType

    nc_ = cfg["NC"]
    shard = cfg["SHARD"]
    ntile_nat = shard // 128
    h = cfg["H"]
    fin = cfg["F_IN"]
    nchunk = cfg["NCHUNK"]
    coh = cfg["COH"]
    ch_a = meta["ch_a"]
    ch_d = meta["ch_d"]
    rows_a = meta["rows_a"]
    rows_d = meta["rows_d"]

    T_total = L["T_total"]
    call_meta = L["call_meta"]
    msg_pos = L["msg_pos"]
    oh_tiles = L["oh_tiles"]
    Tg_max = max(oh_tiles)
    ncoh = len(call_meta)
    idx_cols_total = L["idx16"].shape[1]

    nc = bacc.Bacc(None, target_bir_lowering=False, debug=False, num_devices=nc_)

    iota_in = nc.dram_tensor("iota", [128, Tg_max * 128], fp16,
                             kind="ExternalInput")
    dloc_in = nc.dram_tensor("dloc", [128, T_total], fp16, kind="ExternalInput")
    ddrow_in = nc.dram_tensor("ddrow", [128, G * 128], fp16,
                              kind="ExternalInput")
    idx_in = nc.dram_tensor("idx16", [128, idx_cols_total], i16,
                            kind="ExternalInput")

    if part == "a":
        xT_in = nc.dram_tensor("xT", [fin, shard], fp16, kind="ExternalInput")
        w1_in = nc.dram_tensor("W1", [fin, h], fp16, kind="ExternalInput")
        w2_in = nc.dram_tensor("W2", [h, h], fp16, kind="ExternalInput")
        s1_in = nc.dram_tensor("s1", [h, 1], fp32, kind="ExternalInput")
        t1_in = nc.dram_tensor("t1", [h, 1], fp32, kind="ExternalInput")
        dn_in = nc.dram_tensor("dinv_nat", [128, ntile_nat], fp32,
                               kind="ExternalInput")
        dp_in = nc.dram_tensor("dinv_padlay", [128, G], fp32,
                               kind="ExternalInput")
        out_xs2 = [nc.dram_tensor(f"xs2_out_{q}", [rows_d[q], h], fp16,
                                  kind="ExternalOutput")
                   for q in range(nchunk)]
    else:
        xs2_in = [nc.dram_tensor(f"xs2_in_{q}", [rows_d[q], h], fp16,
                                 kind="ExternalInput")
                  for q in range(nchunk)]
        wf_in = nc.dram_tensor("Wf", [h, 2], fp16, kind="ExternalInput")
        bf_in = nc.dram_tensor("bf_rep", [128, 2], fp32, kind="ExternalInput")
        s2_in = nc.dram_tensor("s2", [h, 1], fp32, kind="ExternalInput")
        t2_in = nc.dram_tensor("t2", [h, 1], fp32, kind="ExternalInput")
        out_lp = nc.dram_tensor("logp", [128, 2 * G], fp32, kind="ExternalOutput")

    with tile.TileContext(nc) as tc:
        with (
            tc.tile_pool(name="res", bufs=1) as res,
            tc.tile_pool(name="big", bufs=1) as big,
            tc.tile_pool(name="stream", bufs=1) as st,
            tc.tile_pool(name="ps", bufs=1, space="PSUM") as ps,
            tc.tile_pool(name="dram", bufs=1, space="DRAM") as dram,
        ):
            iota_t = res.tile([128, Tg_max, 128], fp16)
            dloc_t = res.tile([128, T_total], fp16)
            ddrow_t = res.tile([128, G * 128], fp16)
            idx_t = res.tile([128, idx_cols_total], i16)
            nc.sync.dma_start(out=iota_t[:],
                              in_=iota_in[:].rearrange("p (k d) -> p k d", d=128))
            nc.sync.dma_start(out=dloc_t[:], in_=dloc_in[:])
            nc.sync.dma_start(out=ddrow_t[:], in_=ddrow_in[:])
            nc.sync.dma_start(out=idx_t[:], in_=idx_in[:])

            def edge_layer(tables, s_t, t_t, post_group):
                """tables: list of nchunk DRAM APs (chunk message tables)."""
                ohcol = [0] * G
                acc = 0
                for g in range(G):
                    ohcol[g] = acc
                    acc += oh_tiles[g]
                for hcoh in range(ncoh):
                    gs = list(range(hcoh * coh, min((hcoh + 1) * coh, G)))
                    T_h = sum(oh_tiles[g] for g in gs)
                    msg = st.tile([128, T_h, h], fp16, name="msg", tag="msg",
                                  bufs=2)
                    for (q, col_lo, ncols, ntq, off) in call_meta[hcoh]:
                        nidx = ntq * 128
                        nc.gpsimd.dma_gather(
                            msg[:, off:off + ntq, :],
                            tables[q],
                            idx_t[:, col_lo:col_lo + ncols],
                            nidx,
                            nidx,
                            h,
                        )
                    for g in gs:
                        Tg = oh_tiles[g]
                        oh = st.tile([128, Tg_max, 128], fp16, name="oh",
                                     tag="oh", bufs=3)
                        nc.vector.tensor_tensor(
                            out=oh[:, :Tg, :],
                            in0=iota_t[:, :Tg, :],
                            in1=dloc_t[:, ohcol[g]:ohcol[g] + Tg]
                                .to_broadcast([128, Tg, 128]),
                            op=mybir.AluOpType.is_equal,
                        )
                        pg = ps.tile([h, 128], fp32, name="pg", tag="pg",
                                     bufs=4)
                        for i, tp in enumerate(msg_pos[g]):
                            nc.tensor.matmul(pg[:], msg[:, tp, :],
                                             oh[:, i, :],
                                             start=(i == 0),
                                             stop=(i == Tg - 1))
                        tmp = st.tile([h, 128], fp32, name="tmp", tag="tmp",
                                      bufs=4)
                        nc.vector.tensor_tensor(
                            out=tmp[:], in0=pg[:],
                            in1=ddrow_t[:, g * 128:(g + 1) * 128],
                            op=mybir.AluOpType.mult,
                        )
                        hblk = st.tile([h, 128], fp16, name="hblk",
                                       tag="hblk", bufs=4)
                        nc.scalar.activation(
                            out=hblk[:], in_=tmp[:],
                            func=AF.Relu, bias=t_t[:], scale=s_t[:],
                        )
                        post_group(g, hblk)

            if part == "a":
                w1_t = res.tile([fin, h], fp16)
                w2_t = res.tile([h, h], fp16)
                s1_t = res.tile([h, 1], fp32)
                t1_t = res.tile([h, 1], fp32)
                dn_t = res.tile([128, ntile_nat], fp32)
                dp_t = res.tile([128, G], fp32)
                for t_, i_ in ((w1_t, w1_in), (w2_t, w2_in), (s1_t, s1_in),
                               (t1_t, t1_in), (dn_t, dn_in), (dp_t, dp_in)):
                    nc.sync.dma_start(out=t_[:], in_=i_[:])

                xs1_shard = dram.tile([shard, h], fp16)
                xs1_q = [dram.tile([rows_a[q], h], fp16, addr_space="Shared",
                                   name=f"xs1q{q}")
                         for q in range(nchunk)]

                # ---- stage A: xs1 tiles, staged + AllGather'd per chunk
                xsb = big.tile([128, ntile_nat * 128], fp16, name="xsb",
                               tag="big_a")
                for q, (lo, hi) in enumerate(ch_a):
                    for j in range(lo, hi):
                        lhsT = st.tile([128, 128], fp16, name="xTt",
                                       tag="lhsT", bufs=4)
                        nc.sync.dma_start(
                            out=lhsT[:], in_=xT_in[:, j * 128:(j + 1) * 128])
                        pxs = ps.tile([128, h], fp32, name="pxs", tag="pxs",
                                      bufs=2)
                        nc.tensor.matmul(pxs[:], lhsT[:], w1_t[:], start=True,
                                         stop=True)
                        nc.vector.tensor_scalar(
                            out=xsb[:, j * 128:(j + 1) * 128], in0=pxs[:],
                            scalar1=dn_t[:, j:j + 1], scalar2=None,
                            op0=mybir.AluOpType.mult)
                    rows = hi - lo
                    dest = bass.AP(xs1_shard[:].tensor, lo * 128 * h,
                                   [[h, 128], [128 * h, rows], [1, h]])
                    nc.sync.dma_start(out=dest, in_=xsb[:].rearrange(
                        "p (j f) -> p j f", f=h)[:, lo:hi, :])
                    nc.gpsimd.collective_compute(
                        "AllGather", mybir.AluOpType.bypass,
                        replica_groups=[list(range(nc_))],
                        ins=[xs1_shard[lo * 128:hi * 128, :].opt()],
                        outs=[xs1_q[q][:].opt()],
                    )

                # ---- layer 1 with interleaved xs2 production + AG2
                xs2_shard = dram.tile([G * 128, h], fp16)
                xs2q_int = [dram.tile([rows_d[q], h], fp16,
                                      addr_space="Shared", name=f"xs2qi{q}")
                            for q in range(nchunk)]
                xs2b = big.tile([128, G * 128], fp16, name="xs2b", tag="big_c")
                g_last = {hi - 1: q for q, (lo, hi) in enumerate(ch_d)}

                def post_group(g, hblk):
                    pxs = ps.tile([128, h], fp32, name="pxs2", tag="pxs",
                                  bufs=2)
                    nc.tensor.matmul(pxs[:], hblk[:], w2_t[:], start=True,
                                     stop=True)
                    nc.vector.tensor_scalar(
                        out=xs2b[:, g * 128:(g + 1) * 128], in0=pxs[:],
                        scalar1=dp_t[:, g:g + 1], scalar2=None,
                        op0=mybir.AluOpType.mult)
                    if g in g_last:
                        q = g_last[g]
                        lo, hi = ch_d[q]
                        rows = hi - lo
                        dest = bass.AP(xs2_shard[:].tensor, lo * 128 * h,
                                       [[h, 128], [128 * h, rows], [1, h]])
                        nc.sync.dma_start(out=dest, in_=xs2b[:].rearrange(
                            "p (j f) -> p j f", f=h)[:, lo:hi, :])
                        nc.gpsimd.collective_compute(
                            "AllGather", mybir.AluOpType.bypass,
                            replica_groups=[list(range(nc_))],
                            ins=[xs2_shard[lo * 128:hi * 128, :].opt()],
                            outs=[xs2q_int[q][:].opt()],
                        )
                        nc.sync.dma_start(out=out_xs2[q][:],
                                          in_=xs2q_int[q][:])

                edge_layer([t[:] for t in xs1_q], s1_t, t1_t, post_group)
            else:
                wf_t = res.tile([h, 2], fp16)
                bf_t = res.tile([128, 2], fp32)
                s2_t = res.tile([h, 1], fp32)
                t2_t = res.tile([h, 1], fp32)
                for t_, i_ in ((wf_t, wf_in), (bf_t, bf_in), (s2_t, s2_in),
                               (t2_t, t2_in)):
                    nc.sync.dma_start(out=t_[:], in_=i_[:])

                lg = res.tile([128, 2 * G], fp32)

                def post_group_b(g, hblk):
                    plg = ps.tile([128, 2], fp32, name="plg", tag="plg",
                                  bufs=2)
                    nc.tensor.matmul(plg[:], hblk[:], wf_t[:], start=True,
                                     stop=True)
                    nc.vector.tensor_add(out=lg[:, 2 * g:2 * g + 2],
                                         in0=plg[:], in1=bf_t[:])

                edge_layer([t[:] for t in xs2_in], s2_t, t2_t, post_group_b)

                def strided(base, start):
                    a = base[:]
                    return bass.AP(a.tensor, a.offset + start,
                                   [a.ap[0], [2, G]])

                z0, z1 = strided(lg, 0), strided(lg, 1)
                mx = res.tile([128, G], fp32)
                nc.vector.tensor_tensor(out=mx[:], in0=z0, in1=z1,
                                        op=mybir.AluOpType.max)
                sm0 = res.tile([128, G], fp32)
                sm1 = res.tile([128, G], fp32)
                nc.vector.tensor_sub(out=sm0[:], in0=z0, in1=mx[:])
                nc.vector.tensor_sub(out=sm1[:], in0=z1, in1=mx[:])
                e0 = res.tile([128, G], fp32)
                e1 = res.tile([128, G], fp32)
                nc.scalar.activation(out=e0[:], in_=sm0[:], func=AF.Exp)
                nc.scalar.activation(out=e1[:], in_=sm1[:], func=AF.Exp)
                se = res.tile([128, G], fp32)
                nc.vector.tensor_add(out=se[:], in0=e0[:], in1=e1[:])
                ls = res.tile([128, G], fp32)
                nc.scalar.activation(out=ls[:], in_=se[:], func=AF.Ln)
                nc.vector.tensor_sub(out=sm0[:], in0=sm0[:], in1=ls[:])
                nc.vector.tensor_sub(out=sm1[:], in0=sm1[:], in1=ls[:])
                lpo = res.tile([128, 2 * G], fp32)
                nc.vector.tensor_copy(out=strided(lpo, 0), in_=sm0[:])
                nc.vector.tensor_copy(out=strided(lpo, 1), in_=sm1[:])
                nc.sync.dma_start(out=out_lp[:], in_=lpo[:])

    nc.compile()
    return nc


# ---------------------------------------------------------------- main entry
def _run(x, edge_index, game_indices,
         W1, b1, g1, be1, m1, v1, W2, b2, g2, be2, m2, v2, Wf, bf,
         trace=False, cfg=None):
    from concourse import bass_utils

    if cfg is None:
        cfg = dict(N=N, NPAD=NPAD, SHARD=SHARD, NC=NC, GROUP_EDGES=GROUP_EDGES,
                   H=H, F_IN=F_IN, NCHUNK=NCHUNK, COH=COH, NIDX_CAP=1024)

    x = np.asarray(x, dtype=np.float32)
    key = ("prep", x.shape, int(np.asarray(edge_index)[0, 0]),
           int(np.asarray(edge_index).sum() % (1 << 31)))
    if key in _CACHE:
        per_core, meta, G = _CACHE[key]
    else:
        per_core, meta, G = _prepare(x, np.asarray(edge_index), cfg)
        _CACHE.clear()
        _CACHE[key] = (per_core, meta, G)

    # all cores share the same tile-structure *shapes* only if identical;
    # build per distinct shape signature
    def sig(L):
        return (L["T_total"], L["idx16"].shape[1],
                tuple(tuple(m) for h_ in L["call_meta"] for m in h_),
                tuple(oh for oh in L["oh_tiles"]),
                tuple(tuple(p) for p in L["msg_pos"]))

    bkey_a = ("bass_a", G, sig(per_core[0]["L1"]))
    bkey_b = ("bass_b", G, sig(per_core[0]["L2"]))
    same_a = all(sig(pc["L1"]) == sig(per_core[0]["L1"]) for pc in per_core)
    same_b = all(sig(pc["L2"]) == sig(per_core[0]["L2"]) for pc in per_core)
    assert same_a and same_b, "per-core tile structures differ; SPMD needs one"

    if bkey_a in _CACHE:
        nc_a = _CACHE[bkey_a]
    else:
        nc_a = _build(cfg, G, "a", meta, per_core[0]["L1"])
        _CACHE[bkey_a] = nc_a
    if bkey_b in _CACHE:
        nc_b = _CACHE[bkey_b]
    else:
        nc_b = _build(cfg, G, "b", meta, per_core[0]["L2"])
        _CACHE[bkey_b] = nc_b

    s1, t1 = _fold_bn(np.asarray(g1), np.asarray(be1), np.asarray(m1),
                      np.asarray(v1), np.asarray(b1))
    s2, t2 = _fold_bn(np.asarray(g2), np.asarray(be2), np.asarray(m2),
                      np.asarray(v2), np.asarray(b2))
    Tg_max1 = max(per_core[0]["L1"]["oh_tiles"])
    Tg_max2 = max(per_core[0]["L2"]["oh_tiles"])
    bf_rep = np.broadcast_to(np.asarray(bf, dtype=np.float32), (128, 2)).copy()

    ncores = cfg["NC"]
    in_maps_a = []
    for c in range(ncores):
        pc = per_core[c]
        in_maps_a.append(dict(
            xT=pc["xT"], W1=np.asarray(W1, np.float16),
            W2=np.asarray(W2, np.float16), s1=s1, t1=t1,
            iota=np.tile(np.arange(128, dtype=np.float16), (128, Tg_max1)),
            idx16=pc["L1"]["idx16"], dloc=pc["L1"]["dloc"],
            ddrow=pc["ddrow"], dinv_nat=pc["dinv_nat"],
            dinv_padlay=pc["dinv_padlay"],
        ))
    res_a = bass_utils.run_bass_kernel_spmd(
        nc_a, in_maps_a, core_ids=list(range(ncores)), trace=trace)

    in_maps_b = []
    for c in range(ncores):
        pc = per_core[c]
        m = dict(
            Wf=np.asarray(Wf, np.float16), bf_rep=bf_rep, s2=s2, t2=t2,
            iota=np.tile(np.arange(128, dtype=np.float16), (128, Tg_max2)),
            idx16=pc["L2"]["idx16"], dloc=pc["L2"]["dloc"], ddrow=pc["ddrow"],
        )
        for q in range(cfg["NCHUNK"]):
            m[f"xs2_in_{q}"] = res_a.results[c][f"xs2_out_{q}"]
        in_maps_b.append(m)
    res_b = bass_utils.run_bass_kernel_spmd(
        nc_b, in_maps_b, core_ids=list(range(ncores)), trace=trace)

    class _Res:
        pass

    res = _Res()
    res.results = res_b.results
    res.exec_time_ns = ((res_a.exec_time_ns or 0) + (res_b.exec_time_ns or 0)) \
        if (res_a.exec_time_ns or res_b.exec_time_ns) else None
    res.parts = (res_a, res_b)

    gi = np.asarray(game_indices, dtype=np.int64)
    cji = meta["pad_cji"][gi]
    lp = np.stack([res_b.results[c]["logp"] for c in range(ncores)])
    out = np.empty((gi.shape[0], 2), dtype=np.float32)
    out[:, 0] = lp[cji[:, 0], cji[:, 2], 2 * cji[:, 1]]
    out[:, 1] = lp[cji[:, 0], cji[:, 2], 2 * cji[:, 1] + 1]
    return out, res


def kernel(**inputs):
    out, _ = _run(**inputs)
    return out


def kernel_profiled(**inputs):
    out, res = _run(**inputs, trace=True)
    return out, res
